# revision 6
# baseline (speedup 1.0000x reference)
"""Trainium2 Bass kernel for nn_Loss_26139170963548 (gnn_message_passing).

Math (n=512, d=dy=256, K=16, N=20000, H=512, w=0.7):
  nb[i,j]   = |dist(x_i,x_j) - dist(y_i,y_j)|
  cx[i,j]   = w*x_i + (1-w)*x_j           (same for y)
  nf[j,k]   = dataset[graph[j,k]]
  on[j,k]   = relu(nf @ W1) @ W2
  dx[i,j,k] = dist(cx[i,j], nf[j,k]),  dy = dist(cy[i,j], on[j,k])
  term[i,j] = mean_k |dx - dy|
  ct        = sum_{i != j} term / n^2
  returns (nb.mean(), ct, (nb + ct)/2)

Decomposition (no [n,n,d] tensors):
  |cx_ij - nf_jk|^2 = w^2|x_i|^2 + 2w(1-w) x_i.x_j + (1-w)^2|x_j|^2
                      + |nf_jk|^2 - 2w x_i.nf_jk - 2(1-w) x_j.nf_jk
All i-independent terms fold into a per-column constant; the x_i.nf_jk and
x_i.x_j terms are matmuls accumulated into one PSUM tile, scaled/sqrted by
one ScalarE activation with per-partition bias.

Sharding: j (columns of the pair grid) is split across 8 cores, 64 cols each.
Each core runs the MLP only for its own 64*16=1024 gathered neighbors.
Host does the index gather, transposes, per-column constants, and the final
(nb + ct)/2 assembly; all FLOPs-heavy work runs on the NeuronCores.
"""

import numpy as np
from contextlib import ExitStack

N_CORES = 8
N, D, DY, KNB, H = 512, 256, 256, 16, 512
W = 0.7
JS = N // N_CORES            # 64 j columns per core
JK = JS * KNB                # 1024 neighbor columns per core
NT = N // 128                # 4 i-tiles
NCH = JK // 512              # 2 jk chunks of 512

_CACHE = {}


def _build():
    import concourse.bass as bass
    import concourse.mybir as mybir
    import concourse.tile as tile
    from concourse import bacc

    F32 = mybir.dt.float32
    F32R = mybir.dt.float32r
    AF = mybir.ActivationFunctionType
    ALU = mybir.AluOpType

    nc = bacc.Bacc("TRN2", target_bir_lowering=False, debug=False,
                   enable_asserts=False, num_devices=N_CORES)

    def din(name, shape, dt=F32):
        return nc.dram_tensor(name, shape, dt, kind="ExternalInput").ap()

    xT_d = din("xT", [D, N], F32R)
    yT_d = din("yT", [DY, N], F32R)
    xTs_d = din("xTs", [D, JS], F32R)
    yTs_d = din("yTs", [DY, JS], F32R)
    nfT_d = din("nfT", [D, JK], F32R)
    W1_d = din("W1", [D, H], F32R)
    W2_d = din("W2", [H, DY], F32R)
    xreps_d = din("xreps", [D, JK], F32R)
    yreps_d = din("yreps", [DY, JK], F32R)
    ccx_d = din("ccx", [1, JK], F32R)
    hvy_d = din("hvy", [1, JK])
    ones1_d = din("ones1", [1, 128], F32R)
    onesc_d = din("onesc", [128, 1], F32R)
    nbx_d = din("nbx", [1, JS], F32R)
    nby_d = din("nby", [1, JS], F32R)
    xb2_d = din("xb2", [N, 1])
    yb2_d = din("yb2", [N, 1])
    xb1_d = din("xb1", [N, 1])
    yb1_d = din("yb1", [N, 1])
    xbh_d = din("xbh", [N, 1])
    ybh_d = din("ybh", [N, 1])
    mask_d = din("mask", [N, JS])

    nb_o = nc.dram_tensor("nb_out", [N, JS], F32, kind="ExternalOutput").ap()
    ctv_o = nc.dram_tensor("ctv", [N, 1], F32, kind="ExternalOutput").ap()

    with tile.TileContext(nc) as tc, ExitStack() as ctx:
        cp = ctx.enter_context(tc.tile_pool(name="const", bufs=1))
        wp = ctx.enter_context(tc.tile_pool(name="work", bufs=3))
        dxp = ctx.enter_context(tc.tile_pool(name="dxp", bufs=2))
        mlps = ctx.enter_context(tc.tile_pool(name="mlps", bufs=2, space="PSUM"))
        dxps = ctx.enter_context(tc.tile_pool(name="dxps", bufs=3, space="PSUM"))
        nbps = ctx.enter_context(tc.tile_pool(name="nbps", bufs=1, space="PSUM"))
        stps = ctx.enter_context(tc.tile_pool(name="stps", bufs=2, space="PSUM"))

        def load(name, dram, shape, dt, parts=128):
            tiles = []
            p = shape[0]
            for c in range(0, p, parts):
                h = min(parts, p - c)
                t = cp.tile([h, shape[1]], dt, tag=f"{name}{c}", name=f"{name}{c}")
                nc.sync.dma_start(t[:], dram[c:c + h, :])
                tiles.append(t)
            return tiles

        xT = load("xT", xT_d, [D, N], F32R)
        yT = load("yT", yT_d, [DY, N], F32R)
        xTs = load("xTs", xTs_d, [D, JS], F32R)
        yTs = load("yTs", yTs_d, [DY, JS], F32R)
        nfT = load("nfT", nfT_d, [D, JK], F32R)
        W1 = load("W1", W1_d, [D, H], F32R)
        W2 = load("W2", W2_d, [H, DY], F32R)
        xreps = load("xreps", xreps_d, [D, JK], F32R)
        yreps = load("yreps", yreps_d, [DY, JK], F32R)
        ccx = load("ccx", ccx_d, [1, JK], F32R)[0]
        hvy = load("hvy", hvy_d, [1, JK], F32)[0]
        ones1 = load("ones1", ones1_d, [1, 128], F32R)[0]
        onesc = load("onesc", onesc_d, [128, 1], F32R)[0]
        nbx = load("nbx", nbx_d, [1, JS], F32R)[0]
        nby = load("nby", nby_d, [1, JS], F32R)[0]
        xb2 = load("xb2", xb2_d, [N, 1], F32)
        yb2 = load("yb2", yb2_d, [N, 1], F32)
        xb1 = load("xb1", xb1_d, [N, 1], F32)
        yb1 = load("yb1", yb1_d, [N, 1], F32)
        xbh = load("xbh", xbh_d, [N, 1], F32)
        ybh = load("ybh", ybh_d, [N, 1], F32)
        mask = load("mask", mask_d, [N, JS], F32)

        # ---- MLP over the core's 1024 gathered neighbors (transposed) ----
        # hT[H, r] = relu(W1.T @ nfT);  onT[dy, r] = W2.T @ hT
        hT = [cp.tile([128, JK], F32R, tag=f"hT{p}", name=f"hT{p}") for p in range(4)]
        for p in range(4):
            for q in range(NCH):
                ps = mlps.tile([128, 512], F32, tag="mlp", name="mlp")
                for c in range(2):
                    nc.tensor.matmul(ps[:], W1[c][:, p * 128:(p + 1) * 128],
                                     nfT[c][:, q * 512:(q + 1) * 512],
                                     start=(c == 0), stop=(c == 1))
                nc.scalar.activation(hT[p][:, q * 512:(q + 1) * 512], ps[:], AF.Relu)

        onT = [cp.tile([128, JK], F32R, tag=f"onT{p}", name=f"onT{p}") for p in range(2)]
        onsq = [cp.tile([128, JK], F32R, tag=f"onsq{p}", name=f"onsq{p}") for p in range(2)]
        for p in range(2):
            for q in range(NCH):
                ps = mlps.tile([128, 512], F32, tag="mlp", name="mlp")
                for c in range(4):
                    nc.tensor.matmul(ps[:], W2[c][:, p * 128:(p + 1) * 128],
                                     hT[c][:, q * 512:(q + 1) * 512],
                                     start=(c == 0), stop=(c == 3))
                nc.scalar.activation(onT[p][:, q * 512:(q + 1) * 512], ps[:], AF.Copy)
                nc.scalar.square(onsq[p][:, q * 512:(q + 1) * 512], ps[:])

        # ---- y-side column stats: on2, By -> ccy ----
        # tmp = yreps (.) onT  (= -(1-w) * y_j . on_jk per column)
        tmp = [cp.tile([128, JK], F32R, tag=f"byt{p}", name=f"byt{p}") for p in range(2)]
        for p in range(2):
            nc.vector.tensor_mul(tmp[p][:], yreps[p][:].bitcast(F32),
                                 onT[p][:].bitcast(F32))
        ccy = cp.tile([1, JK], F32R, tag="ccy", name="ccy")
        for q in range(NCH):
            sl = slice(q * 512, (q + 1) * 512)
            on2ps = stps.tile([1, 512], F32, tag="stat", name="stat")
            nc.tensor.matmul(on2ps[:], onesc[:], onsq[0][:, sl], start=True, stop=False)
            nc.tensor.matmul(on2ps[:], onesc[:], onsq[1][:, sl], start=False, stop=True)
            s1ps = stps.tile([1, 512], F32, tag="stat", name="stat")
            nc.tensor.matmul(s1ps[:], onesc[:], tmp[0][:, sl], start=True, stop=False)
            nc.tensor.matmul(s1ps[:], onesc[:], tmp[1][:, sl], start=False, stop=True)
            # ccy = hvy - on2/(2w) - s1/w
            u = wp.tile([1, 512], F32, tag="ccu", name="ccu")
            nc.vector.scalar_tensor_tensor(u[:], s1ps[:], -1.0 / W, hvy[:, sl],
                                           op0=ALU.mult, op1=ALU.add)
            nc.vector.scalar_tensor_tensor(ccy[:, sl], on2ps[:],
                                           -1.0 / (2.0 * W), u[:],
                                           op0=ALU.mult, op1=ALU.add)

        # ---- per i-tile: distances, nb, term ----
        for t in range(NT):
            tsl = slice(t * 128, (t + 1) * 128)
            dx = dxp.tile([128, JK], F32, tag="dx", name="dx")
            dy = dxp.tile([128, JK], F32, tag="dy", name="dy")
            for q in range(NCH):
                sl = slice(q * 512, (q + 1) * 512)
                ps = dxps.tile([128, 512], F32, tag="dist", name="dist")
                nc.tensor.matmul(ps[:], xT[0][:, tsl], nfT[0][:, sl], start=True, stop=False)
                nc.tensor.matmul(ps[:], xT[1][:, tsl], nfT[1][:, sl], start=False, stop=False)
                nc.tensor.matmul(ps[:], xT[0][:, tsl], xreps[0][:, sl], start=False, stop=False)
                nc.tensor.matmul(ps[:], xT[1][:, tsl], xreps[1][:, sl], start=False, stop=False)
                nc.tensor.matmul(ps[:], ones1[:], ccx[:, sl], start=False, stop=True)
                nc.scalar.activation(dx[:, sl], ps[:], AF.Sqrt,
                                     bias=xb2[t][:], scale=-2.0 * W)
                ps2 = dxps.tile([128, 512], F32, tag="dist", name="dist")
                nc.tensor.matmul(ps2[:], yT[0][:, tsl], onT[0][:, sl], start=True, stop=False)
                nc.tensor.matmul(ps2[:], yT[1][:, tsl], onT[1][:, sl], start=False, stop=False)
                nc.tensor.matmul(ps2[:], yT[0][:, tsl], yreps[0][:, sl], start=False, stop=False)
                nc.tensor.matmul(ps2[:], yT[1][:, tsl], yreps[1][:, sl], start=False, stop=False)
                nc.tensor.matmul(ps2[:], ones1[:], ccy[:, sl], start=False, stop=True)
                nc.scalar.activation(dy[:, sl], ps2[:], AF.Sqrt,
                                     bias=yb2[t][:], scale=-2.0 * W)

            # nb: xx/yy over this i-tile's 64 shard columns
            xps = nbps.tile([128, JS], F32, tag="nb", name="nbpsx")
            nc.tensor.matmul(xps[:], xT[0][:, tsl], xTs[0][:], start=True, stop=False)
            nc.tensor.matmul(xps[:], xT[1][:, tsl], xTs[1][:], start=False, stop=False)
            nc.tensor.matmul(xps[:], ones1[:], nbx[:], start=False, stop=True)
            nc.vector.tensor_scalar_min(xps[:], xps[:], xbh[t][:])
            xx = wp.tile([128, JS], F32, tag="xx", name="xx")
            nc.scalar.activation(xx[:], xps[:], AF.Sqrt, bias=xb1[t][:], scale=-2.0)
            yps = nbps.tile([128, JS], F32, tag="nb", name="nbpsy")
            nc.tensor.matmul(yps[:], yT[0][:, tsl], yTs[0][:], start=True, stop=False)
            nc.tensor.matmul(yps[:], yT[1][:, tsl], yTs[1][:], start=False, stop=False)
            nc.tensor.matmul(yps[:], ones1[:], nby[:], start=False, stop=True)
            nc.vector.tensor_scalar_min(yps[:], yps[:], ybh[t][:])
            yy = wp.tile([128, JS], F32, tag="yy", name="yy")
            nc.scalar.activation(yy[:], yps[:], AF.Sqrt, bias=yb1[t][:], scale=-2.0)
            nbs = wp.tile([128, JS], F32, tag="nbs", name="nbs")
            nc.vector.tensor_sub(nbs[:], xx[:], yy[:])
            nba = wp.tile([128, JS], F32, tag="nba", name="nba")
            nc.vector.scalar_tensor_tensor(nba[:], nbs[:], -1.0, nbs[:], op0=ALU.mult, op1=ALU.max)
            nbt = wp.tile([128, JS], F32, tag="nbt", name="nbt")
            nc.vector.tensor_mul(nbt[:], nba[:], mask[t][:])
            nc.sync.dma_start(nb_o[tsl, :], nbt[:])

            # term: mean_k |dx - dy|, masked, row-accumulated
            sub = wp.tile([128, JK], F32, tag="sub", name="sub")
            nc.vector.tensor_sub(sub[:], dx[:], dy[:])
            term = wp.tile([128, JS], F32, tag="term", name="term")
            nc.vector.tensor_reduce(term[:], sub[:].rearrange("p (j k) -> p j k", k=KNB),
                                    axis=mybir.AxisListType.X, op=ALU.add,
                                    apply_absolute_value=True)
            tm = wp.tile([128, JS], F32, tag="tm", name="tm")
            ctv = wp.tile([128, 1], F32, tag="ctv", name="ctv")
            nc.vector.scalar_tensor_tensor(tm[:], term[:], 1.0 / KNB, mask[t][:],
                                           op0=ALU.mult, op1=ALU.mult,
                                           accum_out=ctv[:])
            nc.sync.dma_start(ctv_o[tsl, :], ctv[:])

    nc.compile()
    return nc


def _prep(x, y, dataset, W1, W2, graph):
    w = W
    x = np.ascontiguousarray(np.asarray(x, dtype=np.float32))
    y = np.ascontiguousarray(np.asarray(y, dtype=np.float32))
    dataset = np.asarray(dataset, dtype=np.float32)
    W1 = np.ascontiguousarray(np.asarray(W1, dtype=np.float32))
    W2 = np.ascontiguousarray(np.asarray(W2, dtype=np.float32))
    graph = np.asarray(graph)

    nf = dataset[graph.ravel()]                       # [8192, 256]
    xsq = np.einsum('ij,ij->i', x, x)
    ysq = np.einsum('ij,ij->i', y, y)
    xT = np.ascontiguousarray(x.T)
    yT = np.ascontiguousarray(y.T)

    com = {
        "xT": xT, "yT": yT, "W1": W1, "W2": W2,
        "ones1": np.ones((1, 128), np.float32),
        "onesc": np.ones((128, 1), np.float32),
        "xb2": (w * w * xsq)[:, None].astype(np.float32),
        "yb2": (w * w * ysq)[:, None].astype(np.float32),
        "xb1": xsq[:, None].astype(np.float32),
        "yb1": ysq[:, None].astype(np.float32),
        "xbh": ((xsq - 1e-9) / 2.0)[:, None].astype(np.float32),
        "ybh": ((ysq - 1e-9) / 2.0)[:, None].astype(np.float32),
    }
    in_maps = []
    for c in range(N_CORES):
        sl = slice(c * JS, (c + 1) * JS)
        rows = slice(c * JK, (c + 1) * JK)
        nfc = nf[rows]
        nfT = np.ascontiguousarray(nfc.T)
        xs, ys_ = x[sl], y[sl]
        Bx = np.einsum('jd,jkd->jk', xs, nfc.reshape(JS, KNB, D)).ravel()
        nf2 = np.einsum('ij,ij->i', nfc, nfc)
        ccx = -(0.09 * np.repeat(xsq[sl], KNB) + nf2 - 0.6 * Bx) / (2.0 * w)
        hvy = -(0.09 * np.repeat(ysq[sl], KNB)) / (2.0 * w)
        mask = np.ones((N, JS), np.float32)
        mask[np.arange(c * JS, (c + 1) * JS), np.arange(JS)] = 0.0
        m = dict(com)
        m.update({
            "xTs": np.ascontiguousarray(xs.T), "yTs": np.ascontiguousarray(ys_.T),
            "nfT": nfT,
            "xreps": np.ascontiguousarray(np.repeat(-0.3 * xs.T, KNB, axis=1)),
            "yreps": np.ascontiguousarray(np.repeat(-0.3 * ys_.T, KNB, axis=1)),
            "ccx": ccx[None, :].astype(np.float32),
            "hvy": hvy[None, :].astype(np.float32),
            "nbx": (-xsq[sl] / 2.0)[None, :].astype(np.float32),
            "nby": (-ysq[sl] / 2.0)[None, :].astype(np.float32),
            "mask": mask,
        })
        in_maps.append(m)
    return in_maps


def kernel(x, y, dataset, W1, W2, graph, _trace=False):
    from concourse.bass_utils import run_bass_kernel_spmd

    if "nc" not in _CACHE:
        _CACHE["nc"] = _build()
    nc = _CACHE["nc"]

    in_maps = _prep(x, y, dataset, W1, W2, graph)
    res = run_bass_kernel_spmd(nc, in_maps, core_ids=list(range(N_CORES)),
                               trace=_trace)
    _CACHE["last_result"] = res

    nb = np.empty((N, N), np.float32)
    ct_num = 0.0
    for c, r in enumerate(res.results):
        nb[:, c * JS:(c + 1) * JS] = r["nb_out"]
        ct_num += float(r["ctv"].sum())
    ct = np.float32(ct_num / (N * N))
    nb_mean = np.float32(nb.sum(dtype=np.float64) / (N * N))
    sum_loss = ((nb + ct) / 2.0).astype(np.float32)
    return nb_mean, ct, sum_loss
